# revision 3
# baseline (speedup 1.0000x reference)
"""MiniRocket-style dilated conv features on Trainium2 (Bass/Tile).

Problem: x[16,12,5000] f32, per-dilation ternary weight banks
weights[10,1000,12,9], biases[10,1000].  For each dilation d in
[1,2,...,512]: y = conv1d(x, W_d, rhs_dilation=d, SAME) -> [B,1000,5000];
features are max over time and PPV (mean of y > bias) -> [16, 20000].

Strategy (8 NeuronCores, data-parallel over batch, 2 batches/core):
  - Host zero-pads x to xpb[2,12,9096] (2048 = 4*max_d each side), so the
    108-row shifted stack Xs[(j,c), t] = x[c, t+(j-4)d] for one dilation
    is ONE strided DMA (no edge/zero-fill descriptor swarm).
  - Conv as TensorE matmuls: out[k, t] = sum_r W^T[r, k] * Xs[r, t],
    contract dim 108, M=125 kernels/tile, N=512 cols/matmul -> fp32 PSUM
    tiles of [125, 2048] (4 banks; bufs=2 fills PSUM exactly).
  - Per (dilation, ktile, batch) group, 5000 cols split [2048|2048|904]:
      * ScalarE evicts the two 2048 chunks PSUM f32 -> SBUF fp16 (Copy).
      * VectorE does both reductions on the fp16 copy at 4x: running max
        (tensor_scalar op1=max accum) and PPV count (is_gt vs the
        per-partition bias AP, op1=add accum).
      * VectorE handles the 904 tail fully: fused evict+max at 1x from
        PSUM (writes the fp16 copy as its `out`), then is_gt at 4x.
    This balances ACT (~3.70us/group) and DVE (~3.74us/group); TensorE
    (~2.2us/group) has slack.  No duplicated matmuls.
  - Tiny final merges (reduce over 3 chunk slots; ppv = count/5000) +
    DMA out.

Host-side prep is layout only: fp16 casts, zero-padding x, and the
W -> W^T[(j,c),k] transpose.
"""

import numpy as np

import concourse.bacc as bacc
import concourse.bass as bass
import concourse.mybir as mybir
import concourse.tile as tile
from concourse.bass_utils import run_bass_kernel_spmd

L = 5000
C = 12
KLEN = 9
DILS = [1, 2, 4, 8, 16, 32, 64, 128, 256, 512]
ND = len(DILS)
KPD = 1000
NKT = 8          # kernel tiles per dilation
MT = 125         # kernels per tile (psum partition dim)
NB = 2           # batches per core
NCORES = 8
CONTRACT = C * KLEN  # 108
PAD = 4 * DILS[-1]   # 2048
LP = L + 2 * PAD     # 9096 padded length
MM_N = 512
CHUNKS = [(0, 2048), (2048, 4096), (4096, 5000)]
NCH = len(CHUNKS)
FP16 = mybir.dt.float16
F32 = mybir.dt.float32
ALU = mybir.AluOpType
ACTF = mybir.ActivationFunctionType


def _emit(nc, repeat=1):
    xpb = nc.dram_tensor("xpb", [NB, C, LP], FP16, kind="ExternalInput")
    wt = nc.dram_tensor("wt", [ND, CONTRACT, KPD], FP16, kind="ExternalInput")
    bia = nc.dram_tensor("bia", [MT, ND * NKT], F32, kind="ExternalInput")
    out = nc.dram_tensor("out", [NB, 2 * ND * KPD], F32, kind="ExternalOutput")

    for _rep in range(repeat):
        _emit_body(nc, xpb, wt, bia, out)


def _emit_body(nc, xpb, wt, bia, out):
    with tile.TileContext(nc) as tc:
        with (
            tc.tile_pool(name="const", bufs=1) as constp,
            tc.tile_pool(name="xtp", bufs=2) as xtp,
            tc.tile_pool(name="psp", bufs=2, space="PSUM") as psp,
            tc.tile_pool(name="y16p", bufs=3) as y16p,
            tc.tile_pool(name="finp", bufs=1) as finp,
        ):
            lhsT = constp.tile([CONTRACT, ND * KPD], FP16)
            nc.sync.dma_start(
                lhsT.rearrange("r (d m) -> r d m", d=ND),
                wt.ap().rearrange("d r m -> r d m"),
            )
            posb = constp.tile([MT, ND * NKT], F32)
            nc.sync.dma_start(posb[:, :], bia.ap())
            trash = constp.tile([MT, 2048], FP16)
            slots_mx = [
                constp.tile([MT, ND * NKT * NCH], F32, name=f"smx{b}")
                for b in range(NB)
            ]
            slots_ct = [
                constp.tile([MT, ND * NKT * NCH], F32, name=f"sct{b}")
                for b in range(NB)
            ]

            for di, d in enumerate(DILS):
                # Xs[(j,c), (b,t)] = xpad[b, c, PAD + t + (j-4)d]; one DMA.
                xt = xtp.tile([CONTRACT, NB * L], FP16, tag="xt", name=f"xt{di}")
                for b in range(NB):
                    src = bass.AP(
                        xpb,
                        b * C * LP + PAD - 4 * d,
                        [[d, KLEN], [LP, C], [1, L]],
                    )
                    nc.sync.dma_start(xt[:, b * L : (b + 1) * L], src)
                for kt in range(NKT):
                    lhs = lhsT[:, di * KPD + kt * MT : di * KPD + kt * MT + MT]
                    bcol = di * NKT + kt
                    for b in range(NB):
                        for ch, (c0, c1) in enumerate(CHUNKS):
                            w = c1 - c0
                            scol = bcol * NCH + ch
                            pa = psp.tile(
                                [MT, 2048], F32, tag="pa",
                                name=f"pa{di}_{kt}_{b}_{ch}",
                            )
                            for t in range(c0, c1, MM_N):
                                n = min(MM_N, c1 - t)
                                nc.tensor.matmul(
                                    pa[:, t - c0 : t - c0 + n],
                                    lhs,
                                    xt[:, b * L + t : b * L + t + n],
                                    start=True,
                                    stop=True,
                                )
                            y16 = y16p.tile(
                                [MT, 2048], FP16, tag="y16",
                                name=f"y{di}_{kt}_{b}_{ch}",
                            )
                            if ch < 2:
                                # ScalarE: evict y to fp16 SBUF.
                                nc.scalar.activation(
                                    y16[:, :w], pa[:, :w], ACTF.Copy
                                )
                                # VectorE: running max at 4x.
                                nc.vector.tensor_scalar(
                                    trash[:, :w],
                                    y16[:, :w],
                                    0.0,
                                    None,
                                    op0=ALU.add,
                                    op1=ALU.max,
                                    accum_out=slots_mx[b][:, scol : scol + 1],
                                )
                            else:
                                # VectorE: fused evict+max at 1x from PSUM.
                                nc.vector.tensor_scalar(
                                    y16[:, :w],
                                    pa[:, :w],
                                    0.0,
                                    None,
                                    op0=ALU.add,
                                    op1=ALU.max,
                                    accum_out=slots_mx[b][:, scol : scol + 1],
                                )
                            # VectorE: PPV count (y > bias) at 4x.
                            nc.vector.tensor_scalar(
                                trash[:, :w],
                                y16[:, :w],
                                posb[:, bcol : bcol + 1],
                                None,
                                op0=ALU.is_gt,
                                op1=ALU.add,
                                accum_out=slots_ct[b][:, scol : scol + 1],
                            )

            outv = out.ap().rearrange(
                "bb (d s kt p) -> bb p s d kt", d=ND, s=2, kt=NKT
            )
            for b in range(NB):
                mxr = finp.tile([MT, ND * NKT], F32, name=f"mxr{b}")
                nc.vector.tensor_reduce(
                    mxr[:, :],
                    slots_mx[b].rearrange("p (g c) -> p g c", c=NCH),
                    axis=mybir.AxisListType.X,
                    op=ALU.max,
                )
                ctr = finp.tile([MT, ND * NKT], F32, name=f"ctr{b}")
                nc.vector.tensor_reduce(
                    ctr[:, :],
                    slots_ct[b].rearrange("p (g c) -> p g c", c=NCH),
                    axis=mybir.AxisListType.X,
                    op=ALU.add,
                )
                ppv = finp.tile([MT, ND * NKT], F32, name=f"ppv{b}")
                nc.vector.tensor_scalar(
                    ppv[:, :],
                    ctr[:, :],
                    1.0 / L,
                    None,
                    op0=ALU.mult,
                )
                for di in range(ND):
                    nc.sync.dma_start(
                        outv[b, :, 0, di, :], mxr[:, di * NKT : (di + 1) * NKT]
                    )
                    nc.sync.dma_start(
                        outv[b, :, 1, di, :], ppv[:, di * NKT : (di + 1) * NKT]
                    )


_COMPILED = {}


def get_compiled(repeat=1):
    key = repeat
    if key not in _COMPILED:
        nc = bacc.Bacc(
            "TRN2", target_bir_lowering=False, debug=False, num_devices=NCORES
        )
        _emit(nc, repeat=repeat)
        nc.compile()
        _COMPILED[key] = nc
    return _COMPILED[key]


def make_in_maps(x, weights, biases):
    # W[d,k,c,j] -> wt[d, j*12+c, k], matching the Xs row order (j outer, c inner)
    wtr = np.ascontiguousarray(
        weights.astype(np.float16).transpose(0, 3, 2, 1).reshape(ND, CONTRACT, KPD)
    )
    # bias pre-arranged [kernel-in-tile, dilation*ktile] for a contiguous
    # per-partition DMA; positive (used as the is_gt threshold).
    bia = np.ascontiguousarray(
        biases.astype(np.float32).reshape(ND, NKT, MT).transpose(2, 0, 1).reshape(MT, ND * NKT)
    )
    xh = x.astype(np.float16)
    maps = []
    for c in range(NCORES):
        xpb = np.zeros((NB, C, LP), np.float16)
        xpb[:, :, PAD : PAD + L] = xh[NB * c : NB * (c + 1)]
        maps.append({"xpb": xpb, "wt": wtr, "bia": bia})
    return maps


def run(x, weights, biases, trace=False, **kw):
    nc = get_compiled()
    res = run_bass_kernel_spmd(
        nc, make_in_maps(x, weights, biases), core_ids=list(range(NCORES)),
        trace=trace, **kw
    )
    outs = np.concatenate([r["out"] for r in res.results], axis=0)
    return outs.astype(np.float32), res


def kernel(x, weights, biases):
    out, _ = run(x, weights, biases)
    return out


def bench(x, weights, biases, iters=20, repeat=1):
    """Time the sharded PJRT executable with pre-staged device inputs.

    Returns (out, per_call_wall_ns_list). Mirrors bass2jax.run_bass_via_pjrt's
    multi-core path, but stages inputs once and times repeated dispatches.
    """
    import time

    import jax
    from jax.sharding import Mesh, NamedSharding, PartitionSpec
    from jax.experimental.shard_map import shard_map

    import concourse.bass2jax as b2j
    import concourse.mybir as mb

    nc = get_compiled(repeat=repeat)
    b2j.install_neuronx_cc_hook()
    in_maps = make_in_maps(x, weights, biases)

    partition_name = nc.partition_id_tensor.name if nc.partition_id_tensor else None
    in_names, out_names, out_avals, zero_outs = [], [], [], []
    for alloc in nc.m.functions[0].allocations:
        if not isinstance(alloc, mb.MemoryLocationSet):
            continue
        name = alloc.memorylocations[0].name
        if alloc.kind == "ExternalInput":
            if name != partition_name:
                in_names.append(name)
        elif alloc.kind == "ExternalOutput":
            out_names.append(name)
            shape = tuple(alloc.tensor_shape)
            dtype = mb.dt.np(alloc.dtype)
            out_avals.append(jax.core.ShapedArray(shape, dtype))
            zero_outs.append(np.zeros(shape, dtype))
    n_params = len(in_names)
    n_outs = len(out_avals)
    all_names = in_names + out_names
    if partition_name is not None:
        all_names = all_names + [partition_name]

    def _body(*args):
        operands = list(args)
        if partition_name is not None:
            operands.append(b2j.partition_id_tensor())
        outs = b2j._bass_exec_p.bind(
            *operands,
            out_avals=tuple(out_avals),
            in_names=tuple(all_names),
            out_names=tuple(out_names),
            lowering_input_output_aliases=(),
            sim_require_finite=True,
            sim_require_nnan=True,
            nc=nc,
        )
        return tuple(outs)

    devices = jax.devices()[:NCORES]
    mesh = Mesh(np.asarray(devices), ("core",))
    spec = PartitionSpec("core")
    sharded = jax.jit(
        shard_map(
            _body,
            mesh=mesh,
            in_specs=(spec,) * (n_params + n_outs),
            out_specs=(spec,) * n_outs,
            check_rep=False,
        ),
        donate_argnums=tuple(range(n_params, n_params + n_outs)),
        keep_unused=True,
    )
    sh = NamedSharding(mesh, spec)
    concat_in = [
        jax.device_put(
            np.concatenate([np.asarray(m[name]) for m in in_maps], axis=0), sh
        )
        for name in in_names
    ]
    zero_host = [np.zeros((NCORES * z.shape[0], *z.shape[1:]), z.dtype) for z in zero_outs]

    times = []
    out_arrs = None
    for i in range(iters + 1):
        zeros_dev = [jax.device_put(z, sh) for z in zero_host]
        jax.block_until_ready(zeros_dev)
        t0 = time.perf_counter()
        out_arrs = sharded(*concat_in, *zeros_dev)
        jax.block_until_ready(out_arrs)
        t1 = time.perf_counter()
        if i > 0:  # skip warmup/compile call
            times.append((t1 - t0) * 1e9)
    out = np.asarray(out_arrs[out_names.index("out")]).reshape(NCORES * NB, -1)
    return out.astype(np.float32), times


# revision 4
# speedup vs baseline: 1.0252x; 1.0252x over previous
"""MiniRocket-style dilated conv features on Trainium2 (Bass/Tile).

Problem: x[16,12,5000] f32, per-dilation ternary weight banks
weights[10,1000,12,9], biases[10,1000].  For each dilation d in
[1,2,...,512]: y = conv1d(x, W_d, rhs_dilation=d, SAME) -> [B,1000,5000];
features are max over time and PPV (mean of y > bias) -> [16, 20000].

Strategy (8 NeuronCores, data-parallel over batch, 2 batches/core):
  - Host zero-pads x to xpb[2,12,9096] (2048 = 4*max_d each side), so the
    108-row shifted stack Xs[(j,c), t] = x[c, t+(j-4)d] for one dilation
    is ONE strided DMA per batch (no edge/zero-fill descriptor swarm).
  - Conv as TensorE matmuls: out[k, t] = sum_r W^T[r, k] * Xs[r, t],
    contract dim 108, M=125 kernels/tile, N=512 cols/matmul -> fp32 PSUM
    tiles of [125, 2048] (4 banks; bufs=2 fills PSUM exactly).
  - Both reductions read the SAME PSUM tile (single copy, no duplicated
    matmuls; cross-engine reads pipeline fine — measured ~2.28us per
    2048-chunk steady state):
      * ScalarE: PPV via Sign activation (bias = -b) + accum sum.
      * VectorE: running max via tensor_scalar (op1=max) accum.
    All DVE/ACT paths measured ~1x elem/lane/cycle on HW (no 2x/4x
    modes materialize for accum ops), so big chunks (2048) amortize the
    per-instruction fixed cost and the engines split the two passes.
  - Tiny final merges (reduce over 3 chunk slots; ppv = (sum+L)/(2L)) +
    DMA out.

Host-side prep is layout only: fp16 casts, zero-padding x, and the
W -> W^T[(j,c),k] transpose.
"""

import numpy as np

import concourse.bacc as bacc
import concourse.bass as bass
import concourse.mybir as mybir
import concourse.tile as tile
from concourse.bass_utils import run_bass_kernel_spmd

L = 5000
C = 12
KLEN = 9
DILS = [1, 2, 4, 8, 16, 32, 64, 128, 256, 512]
ND = len(DILS)
KPD = 1000
NKT = 8          # kernel tiles per dilation
MT = 125         # kernels per tile (psum partition dim)
NB = 2           # batches per core
NCORES = 8
CONTRACT = C * KLEN  # 108
PAD = 4 * DILS[-1]   # 2048
LP = L + 2 * PAD     # 9096 padded length
MM_N = 512
CHUNKS = [(0, 2048), (2048, 4096), (4096, 5000)]
NCH = len(CHUNKS)
FP16 = mybir.dt.float16
F32 = mybir.dt.float32
ALU = mybir.AluOpType
ACTF = mybir.ActivationFunctionType


def _emit(nc, repeat=1):
    xpb = nc.dram_tensor("xpb", [NB, C, LP], FP16, kind="ExternalInput")
    wt = nc.dram_tensor("wt", [ND, CONTRACT, KPD], FP16, kind="ExternalInput")
    bia = nc.dram_tensor("bia", [MT, ND * NKT], F32, kind="ExternalInput")
    out = nc.dram_tensor("out", [NB, 2 * ND * KPD], F32, kind="ExternalOutput")

    for _rep in range(repeat):
        _emit_body(nc, xpb, wt, bia, out)


def _emit_body(nc, xpb, wt, bia, out):
    with tile.TileContext(nc) as tc:
        with (
            tc.tile_pool(name="const", bufs=1) as constp,
            tc.tile_pool(name="xtp", bufs=2) as xtp,
            tc.tile_pool(name="psp", bufs=2, space="PSUM") as psp,
            tc.tile_pool(name="finp", bufs=1) as finp,
        ):
            lhsT = constp.tile([CONTRACT, ND * KPD], FP16)
            nc.sync.dma_start(
                lhsT.rearrange("r (d m) -> r d m", d=ND),
                wt.ap().rearrange("d r m -> r d m"),
            )
            negb = constp.tile([MT, ND * NKT], F32)
            nc.sync.dma_start(negb[:, :], bia.ap())
            trash_a = constp.tile([MT, 2048], FP16)
            trash_v = constp.tile([MT, 2048], FP16)
            slots_mx = [
                constp.tile([MT, ND * NKT * NCH], F32, name=f"smx{b}")
                for b in range(NB)
            ]
            slots_sg = [
                constp.tile([MT, ND * NKT * NCH], F32, name=f"ssg{b}")
                for b in range(NB)
            ]

            for di, d in enumerate(DILS):
                # Xs[(j,c), (b,t)] = xpad[b, c, PAD + t + (j-4)d]; one DMA/batch.
                xt = xtp.tile([CONTRACT, NB * L], FP16, tag="xt", name=f"xt{di}")
                for b in range(NB):
                    src = bass.AP(
                        xpb,
                        b * C * LP + PAD - 4 * d,
                        [[d, KLEN], [LP, C], [1, L]],
                    )
                    nc.sync.dma_start(xt[:, b * L : (b + 1) * L], src)
                for kt in range(NKT):
                    lhs = lhsT[:, di * KPD + kt * MT : di * KPD + kt * MT + MT]
                    bcol = di * NKT + kt
                    for b in range(NB):
                        for ch, (c0, c1) in enumerate(CHUNKS):
                            w = c1 - c0
                            scol = bcol * NCH + ch
                            pa = psp.tile(
                                [MT, 2048], F32, tag="pa",
                                name=f"pa{di}_{kt}_{b}_{ch}",
                            )
                            for t in range(c0, c1, MM_N):
                                n = min(MM_N, c1 - t)
                                nc.tensor.matmul(
                                    pa[:, t - c0 : t - c0 + n],
                                    lhs,
                                    xt[:, b * L + t : b * L + t + n],
                                    start=True,
                                    stop=True,
                                )
                            # ScalarE: PPV via sign(y - b), accumulated sum.
                            nc.scalar.activation(
                                trash_a[:, :w],
                                pa[:, :w],
                                ACTF.Sign,
                                bias=negb[:, bcol : bcol + 1],
                                accum_out=slots_sg[b][:, scol : scol + 1],
                            )
                            # VectorE: running max via accum reduce.
                            nc.vector.tensor_scalar(
                                trash_v[:, :w],
                                pa[:, :w],
                                0.0,
                                None,
                                op0=ALU.add,
                                op1=ALU.max,
                                accum_out=slots_mx[b][:, scol : scol + 1],
                            )

            outv = out.ap().rearrange(
                "bb (d s kt p) -> bb p s d kt", d=ND, s=2, kt=NKT
            )
            for b in range(NB):
                mxr = finp.tile([MT, ND * NKT], F32, name=f"mxr{b}")
                nc.vector.tensor_reduce(
                    mxr[:, :],
                    slots_mx[b].rearrange("p (g c) -> p g c", c=NCH),
                    axis=mybir.AxisListType.X,
                    op=ALU.max,
                )
                sgr = finp.tile([MT, ND * NKT], F32, name=f"sgr{b}")
                nc.vector.tensor_reduce(
                    sgr[:, :],
                    slots_sg[b].rearrange("p (g c) -> p g c", c=NCH),
                    axis=mybir.AxisListType.X,
                    op=ALU.add,
                )
                # ppv = (#gt)/L = (sum_sign + L)/(2L) = sum_sign/(2L) + 0.5
                ppv = finp.tile([MT, ND * NKT], F32, name=f"ppv{b}")
                nc.vector.tensor_scalar(
                    ppv[:, :],
                    sgr[:, :],
                    1.0 / (2.0 * L),
                    0.5,
                    op0=ALU.mult,
                    op1=ALU.add,
                )
                for di in range(ND):
                    nc.sync.dma_start(
                        outv[b, :, 0, di, :], mxr[:, di * NKT : (di + 1) * NKT]
                    )
                    nc.sync.dma_start(
                        outv[b, :, 1, di, :], ppv[:, di * NKT : (di + 1) * NKT]
                    )


_COMPILED = {}


def get_compiled(repeat=1):
    key = repeat
    if key not in _COMPILED:
        nc = bacc.Bacc(
            "TRN2", target_bir_lowering=False, debug=False, num_devices=NCORES
        )
        _emit(nc, repeat=repeat)
        nc.compile()
        _COMPILED[key] = nc
    return _COMPILED[key]


def make_in_maps(x, weights, biases):
    # W[d,k,c,j] -> wt[d, j*12+c, k], matching the Xs row order (j outer, c inner)
    wtr = np.ascontiguousarray(
        weights.astype(np.float16).transpose(0, 3, 2, 1).reshape(ND, CONTRACT, KPD)
    )
    # negated bias (Sign activation bias), pre-arranged [kernel-in-tile,
    # dilation*ktile] for a contiguous per-partition DMA
    bia = np.ascontiguousarray(
        (-biases.astype(np.float32)).reshape(ND, NKT, MT).transpose(2, 0, 1).reshape(MT, ND * NKT)
    )
    xh = x.astype(np.float16)
    maps = []
    for c in range(NCORES):
        xpb = np.zeros((NB, C, LP), np.float16)
        xpb[:, :, PAD : PAD + L] = xh[NB * c : NB * (c + 1)]
        maps.append({"xpb": xpb, "wt": wtr, "bia": bia})
    return maps


def run(x, weights, biases, trace=False, **kw):
    nc = get_compiled()
    res = run_bass_kernel_spmd(
        nc, make_in_maps(x, weights, biases), core_ids=list(range(NCORES)),
        trace=trace, **kw
    )
    outs = np.concatenate([r["out"] for r in res.results], axis=0)
    return outs.astype(np.float32), res


def kernel(x, weights, biases):
    out, _ = run(x, weights, biases)
    return out


def bench(x, weights, biases, iters=20, repeat=1):
    """Time the sharded PJRT executable with pre-staged device inputs.

    Returns (out, per_call_wall_ns_list). Mirrors bass2jax.run_bass_via_pjrt's
    multi-core path, but stages inputs once and times repeated dispatches.
    """
    import time

    import jax
    from jax.sharding import Mesh, NamedSharding, PartitionSpec
    from jax.experimental.shard_map import shard_map

    import concourse.bass2jax as b2j
    import concourse.mybir as mb

    nc = get_compiled(repeat=repeat)
    b2j.install_neuronx_cc_hook()
    in_maps = make_in_maps(x, weights, biases)

    partition_name = nc.partition_id_tensor.name if nc.partition_id_tensor else None
    in_names, out_names, out_avals, zero_outs = [], [], [], []
    for alloc in nc.m.functions[0].allocations:
        if not isinstance(alloc, mb.MemoryLocationSet):
            continue
        name = alloc.memorylocations[0].name
        if alloc.kind == "ExternalInput":
            if name != partition_name:
                in_names.append(name)
        elif alloc.kind == "ExternalOutput":
            out_names.append(name)
            shape = tuple(alloc.tensor_shape)
            dtype = mb.dt.np(alloc.dtype)
            out_avals.append(jax.core.ShapedArray(shape, dtype))
            zero_outs.append(np.zeros(shape, dtype))
    n_params = len(in_names)
    n_outs = len(out_avals)
    all_names = in_names + out_names
    if partition_name is not None:
        all_names = all_names + [partition_name]

    def _body(*args):
        operands = list(args)
        if partition_name is not None:
            operands.append(b2j.partition_id_tensor())
        outs = b2j._bass_exec_p.bind(
            *operands,
            out_avals=tuple(out_avals),
            in_names=tuple(all_names),
            out_names=tuple(out_names),
            lowering_input_output_aliases=(),
            sim_require_finite=True,
            sim_require_nnan=True,
            nc=nc,
        )
        return tuple(outs)

    devices = jax.devices()[:NCORES]
    mesh = Mesh(np.asarray(devices), ("core",))
    spec = PartitionSpec("core")
    sharded = jax.jit(
        shard_map(
            _body,
            mesh=mesh,
            in_specs=(spec,) * (n_params + n_outs),
            out_specs=(spec,) * n_outs,
            check_rep=False,
        ),
        donate_argnums=tuple(range(n_params, n_params + n_outs)),
        keep_unused=True,
    )
    sh = NamedSharding(mesh, spec)
    concat_in = [
        jax.device_put(
            np.concatenate([np.asarray(m[name]) for m in in_maps], axis=0), sh
        )
        for name in in_names
    ]
    zero_host = [np.zeros((NCORES * z.shape[0], *z.shape[1:]), z.dtype) for z in zero_outs]

    times = []
    out_arrs = None
    for i in range(iters + 1):
        zeros_dev = [jax.device_put(z, sh) for z in zero_host]
        jax.block_until_ready(zeros_dev)
        t0 = time.perf_counter()
        out_arrs = sharded(*concat_in, *zeros_dev)
        jax.block_until_ready(out_arrs)
        t1 = time.perf_counter()
        if i > 0:  # skip warmup/compile call
            times.append((t1 - t0) * 1e9)
    out = np.asarray(out_arrs[out_names.index("out")]).reshape(NCORES * NB, -1)
    return out.astype(np.float32), times
